# revision 35
# baseline (speedup 1.0000x reference)
"""Gemma3 sliding-window attention layer on 8 Trainium2 NeuronCores.

Tensor-parallel over query heads: core h computes query head h (kv head
h//2), i.e. column-parallel qkv projection, full per-head attention, and
the row-parallel slice of o_proj; the 8 partial [S, H] bf16 outputs are
summed on the host in f32 (the all-reduce / unshard step).

Layout: hidden_states transposed on the host ([H, S]) so q and k come
out of the projection directly in [d, seq] layout (what the scores
matmul needs) and v comes out natural [seq, d] (what attn@v needs).
The softmax denominator rides as a ones-column appended to v.

Schedule (software-pipelined so the PE never idles):
  proj g0 | proj g1 | attn r0-3 | proj g2 | attn r4-7 | proj g3 |
  attn r8-15
Attention rows for group g are enqueued after projection of group g+1,
so the PE chews on proj matmuls while group g's norm + rope finish on
Scalar/Vector, and on attention matmuls while the next group's DMAs
land.  DMA issue is split across the two HWDGE queues (sync + scalar)
to halve head-of-kernel issue serialization.

RMSNorm: the (1+w) gains are folded into wq/wk columns on the host; the
square-sum matmuls use per-partition-weighted lhsT (W2 = (1+w_p)^-2
replicated along free) instead of ones, so the stats still measure the
pre-gain q/k.  rstd comes from a Sqrt activation + fast-approx DVE
reciprocal.  Rope tables are then plain cos/sin (the HF duplicated-half
convention means only the first 128 rows are needed), loaded once.
"""

import numpy as np
import ml_dtypes

import concourse.bass as bass
import concourse.mybir as mybir
import concourse.tile as tile
from concourse.bass_utils import run_bass_kernel_spmd
from concourse.masks import make_identity

# ---- problem constants (hardcoded; kernel.py must be self-contained) ----
S = 2048          # sequence length
H = 2560          # hidden size
NH = 8            # query heads
NKV = 4           # kv heads
D = 256           # head dim
EPS = 1e-6
SCALING = 256.0 ** -0.5
WINDOW = 1024 - 1  # sliding window - 1

N_CORES = 8
KC = H // 128      # 20 contraction chunks for the projection
RB = S // 128      # 16 row blocks
GW = 512           # column group width
NG = S // GW       # 4 groups
MASK_VAL = -1e10
BLK_WIN = WINDOW // 128 + 1   # 8: c in [r-8, r] can contribute

F32 = mybir.dt.float32
BF16 = mybir.dt.bfloat16


def _bf16(x):
    return np.ascontiguousarray(x.astype(ml_dtypes.bfloat16))


def _chunk_part(x, p=128):
    """[c*p, n] -> [p, c, n] host relayout so it DMAs 1:1 into an SBUF tile."""
    c = x.shape[0] // p
    return np.ascontiguousarray(
        x.reshape(c, p, *x.shape[1:]).transpose(1, 0, *range(2, x.ndim + 1))
    )


def split_multiwaits(nc):
    """This toolchain's codegen allows one sync-wait slot per instruction.

    Tile emits several waits on the first consumer of multi-queue DMAs and
    on kernel-tail drains; hoist all but the last wait onto same-engine
    NoOps inserted immediately before the offending instruction (queue
    order on the engine preserves the semantics exactly).
    """
    k = 0
    for f in nc.m.functions:
        for bb in f.blocks:
            insts = bb.instructions
            if not any(i.sync_info and len(i.sync_info.on_wait) > 1
                       for i in insts):
                continue
            newlist = []
            for inst in insts:
                si = inst.sync_info
                if si is not None and len(si.on_wait) > 1:
                    for w in list(si.on_wait)[:-1]:
                        nop = mybir.InstNoOp(name=f"{inst.name}-ws{k}")
                        k += 1
                        nop.engine = inst.engine
                        nop.sync_info = mybir.SyncInfo(on_wait=[w], on_update=[])
                        newlist.append(nop)
                    inst.sync_info = mybir.SyncInfo(
                        on_wait=[list(si.on_wait)[-1]],
                        on_update=list(si.on_update))
                newlist.append(inst)
            live = bb.instructions
            live.clear()
            live.extend(newlist)
    return nc


def build_nc():
    """One-core SPMD program (all cores run this; data differs per core)."""
    nc = bass.Bass()

    hT_d = nc.declare_dram_parameter("hT", [128, NG, KC, GW], BF16, isOutput=False)
    wq_d = nc.declare_dram_parameter("wq", [128, KC, D], BF16, isOutput=False)
    wk_d = nc.declare_dram_parameter("wk", [128, KC, D], BF16, isOutput=False)
    wv_d = nc.declare_dram_parameter("wv", [128, KC, D], BF16, isOutput=False)
    cs_d = nc.declare_dram_parameter("cs", [128, 2, S], BF16, isOutput=False)
    w2_d = nc.declare_dram_parameter("w2", [128, 4, 128], BF16, isOutput=False)
    wo_d = nc.declare_dram_parameter("wo", [128, 2, H], BF16, isOutput=False)
    mk_d = nc.declare_dram_parameter("maskT", [128, 2, 128], BF16, isOutput=False)
    out_d = nc.declare_dram_parameter("out", [S, H], BF16, isOutput=True)

    with tile.TileContext(nc) as tc:
        with (
            tc.tile_pool(name="persist", bufs=1) as persist,
            tc.tile_pool(name="ps512", bufs=3, space="PSUM") as ps512,
            tc.tile_pool(name="ps257", bufs=2, space="PSUM") as ps257,
        ):
            # persistent tensors
            v_aug = persist.tile([128, RB, D + 1], BF16, tag="vaug")
            qTf = persist.tile([128, 2, S], BF16, tag="qTf")   # roped+scaled
            kTf = persist.tile([128, 2, S], BF16, tag="kTf")   # roped, unscaled
            cs_sb = persist.tile([128, 2, S], BF16, tag="cs")  # cos / sin
            w2_sb = persist.tile([128, 4, 128], BF16, tag="w2")
            maskT = persist.tile([128, 2, 128], BF16, tag="maskT")
            ident = persist.tile([128, 128], BF16, tag="ident")

            wq_sb = persist.tile([128, KC, D], BF16, tag="wq")
            wk_sb = persist.tile([128, KC, D], BF16, tag="wk")
            wv_sb = persist.tile([128, KC, D], BF16, tag="wv")
            wo_sb = persist.tile([128, 2, H], BF16, tag="wo")

            with (
                tc.tile_pool(name="ht", bufs=2) as htpool,
                tc.tile_pool(name="raw", bufs=2) as rawpool,
                tc.tile_pool(name="sq", bufs=2) as sqpool,
                tc.tile_pool(name="rst", bufs=2) as rstpool,
                tc.tile_pool(name="tmp", bufs=2) as tmppool,
                tc.tile_pool(name="probs", bufs=6) as prpool,
                tc.tile_pool(name="asmall", bufs=4) as aspool,
                tc.tile_pool(name="attnT", bufs=2) as atpool,
                tc.tile_pool(name="opool", bufs=3) as opool,
                tc.tile_pool(name="po", bufs=2, space="PSUM") as po,
                tc.tile_pool(name="psb", bufs=1, space="PSUM") as psb,
            ):
                # ---- head DMAs: first-matmul inputs first, split across
                # ---- the two HWDGE queues (sync gets wq, scalar gets hT g0)
                ht_tiles = [None] * NG
                ht_tiles[0] = htpool.tile([128, KC, GW], BF16, tag="ht",
                                          name="ht0")
                # scalar queue: dedicated to the critical hT g0 stream;
                # sync queue: wq chunks interleaved with hT's tail chunks
                ht0 = ht_tiles[0]
                nc.sync.dma_start(out=wq_sb[:, 0:2, :], in_=wq_d[:, 0:2, :])
                for c0, c1 in ((0, 2), (2, 5), (5, 8), (8, 11), (11, 14)):
                    nc.scalar.dma_start(out=ht0[:, c0:c1, :],
                                        in_=hT_d[:, 0, c0:c1, :])
                nc.sync.dma_start(out=wq_sb[:, 2:8, :], in_=wq_d[:, 2:8, :])
                nc.sync.dma_start(out=wk_sb[:, 0:4, :], in_=wk_d[:, 0:4, :])
                nc.sync.dma_start(out=ht0[:, 14:17, :],
                                  in_=hT_d[:, 0, 14:17, :])
                nc.sync.dma_start(out=wq_sb[:, 8:14, :], in_=wq_d[:, 8:14, :])
                nc.sync.dma_start(out=wk_sb[:, 4:12, :], in_=wk_d[:, 4:12, :])
                nc.sync.dma_start(out=ht0[:, 17:20, :],
                                  in_=hT_d[:, 0, 17:20, :])
                nc.sync.dma_start(out=wq_sb[:, 14:20, :],
                                  in_=wq_d[:, 14:20, :])
                nc.sync.dma_start(out=wk_sb[:, 12:20, :],
                                  in_=wk_d[:, 12:20, :])
                nc.sync.dma_start(out=wv_sb, in_=wv_d[:])
                nc.sync.dma_start(out=cs_sb, in_=cs_d[:])
                nc.sync.dma_start(out=w2_sb, in_=w2_d[:])
                nc.sync.dma_start(out=maskT, in_=mk_d[:])

                make_identity(nc, ident)
                nc.vector.memset(v_aug[:, :, D:D + 1], 1.0)
                eps_q = persist.tile([128, 1], F32, tag="eps_q")
                eps_k = persist.tile([128, 1], F32, tag="eps_k")
                nc.vector.memset(eps_q, EPS / (SCALING * SCALING))
                nc.vector.memset(eps_k, EPS)
                # warm the ln/exp activation table during the DMA-bound
                # head so the 1.3us ACT_TABLE_LOAD never blocks the
                # stats->rope chain mid-pipeline
                warm = persist.tile([128, 1], F32, tag="warm")
                nc.scalar.activation(
                    out=warm, in_=eps_k,
                    func=mybir.ActivationFunctionType.Ln)

                def attn_row(r):
                    """Scores -> exp -> attn@v -> transpose -> o_proj for
                    one 128-token query row block."""
                    cmin = max(0, r - BLK_WIN)
                    cols = list(range(cmin, r + 1))
                    ps_at = ps257.tile([128, D + 1], F32, tag="ps257")
                    chunks = [cols[i0:i0 + 4] for i0 in range(0, len(cols), 4)]
                    # all score matmuls first: exps overlap later chunks'
                    # scores, so the attn@v accumulation never stalls PE
                    psWs = []
                    for chunk in chunks:
                        psW = ps512.tile([128, 512], F32, tag="ps512")
                        for j, c in enumerate(chunk):
                            sl = slice(j * 128, (j + 1) * 128)
                            masked = (c == r) or (c == r - BLK_WIN)
                            for dc in range(2):
                                nc.tensor.matmul(
                                    psW[:, sl],
                                    lhsT=kTf[:, dc, c * 128:(c + 1) * 128],
                                    rhs=qTf[:, dc, r * 128:(r + 1) * 128],
                                    start=(dc == 0),
                                    stop=(dc == 1) and not masked,
                                )
                            if masked:
                                nc.tensor.matmul(
                                    psW[:, sl],
                                    lhsT=maskT[:, 0, :] if c == r
                                    else maskT[:, 1, :],
                                    rhs=ident,
                                    start=False, stop=True,
                                )
                        psWs.append(psW)
                    pTs = []
                    for chunk, psW in zip(chunks, psWs):
                        pT = prpool.tile([128, 512], BF16, tag="pT")
                        w = len(chunk) * 128
                        nc.scalar.activation(
                            out=pT[:, 0:w], in_=psW[:, 0:w],
                            func=mybir.ActivationFunctionType.Exp,
                        )
                        pTs.append(pT)
                    for chunk, pT in zip(chunks, pTs):
                        for j, c in enumerate(chunk):
                            sl = slice(j * 128, (j + 1) * 128)
                            nc.tensor.matmul(
                                ps_at,
                                lhsT=pT[:, sl],
                                rhs=v_aug[:, c, :],
                                start=(c == cmin), stop=(c == r),
                            )
                    rc = aspool.tile([128, 1], F32, tag="rc")
                    nc.vector.reciprocal(rc, ps_at[:, D:D + 1])
                    a_sb = aspool.tile([128, D], BF16, tag="asb")
                    nc.vector.tensor_copy(a_sb, ps_at[:, 0:D])
                    attnT = atpool.tile([128, 2, 128], BF16, tag="attnT")
                    for dc in range(2):
                        pt = psb.tile([128, 128], BF16, tag="psb")
                        nc.tensor.transpose(
                            pt, a_sb[:, dc * 128:(dc + 1) * 128], ident
                        )
                        nc.vector.tensor_copy(attnT[:, dc, :], pt)

                    # o_proj for this row; denom recip folded into the drain
                    o_sb = opool.tile([128, H], BF16, tag="osb")
                    for hc in range(H // 512):
                        ps = po.tile([128, 512], F32, tag="po")
                        for dc in range(2):
                            nc.tensor.matmul(
                                ps,
                                lhsT=attnT[:, dc, :],
                                rhs=wo_sb[:, dc, hc * 512:(hc + 1) * 512],
                                start=(dc == 0), stop=(dc == 1),
                            )
                        if hc in (0, 3):
                            nc.scalar.mul(o_sb[:, hc * 512:(hc + 1) * 512],
                                          ps, rc)
                        else:
                            with nc.allow_low_precision(
                                    reason="bf16 out slice; host sums in f32"):
                                nc.vector.tensor_scalar_mul(
                                    o_sb[:, hc * 512:(hc + 1) * 512], ps, rc)
                    splits = ((0, 512), (512, 1024), (1024, 1536),
                              (1536, 2048), (2048, H)) if r == 15 else \
                             ((0, 1024), (1024, 2048), (2048, H))
                    for h0, h1 in splits:
                        nc.sync.dma_start(
                            out=out_d[r * 128:(r + 1) * 128, h0:h1],
                            in_=o_sb[:, h0:h1])

                for g in range(NG):
                    gsl = slice(g * GW, (g + 1) * GW)
                    ht = ht_tiles[g]
                    if g + 1 < NG:
                        # prefetch next group's hidden slice (both queues)
                        nxt = htpool.tile([128, KC, GW], BF16, tag="ht",
                                          name=f"ht{g + 1}")
                        ht_tiles[g + 1] = nxt
                        nc.sync.dma_start(out=nxt[:, 0:10, :],
                                          in_=hT_d[:, g + 1, 0:10, :])
                        nc.scalar.dma_start(out=nxt[:, 10:20, :],
                                            in_=hT_d[:, g + 1, 10:20, :])
                    if g == 1:
                        for dc in range(2):
                            nc.sync.dma_start(out=wo_sb[:, dc, :],
                                              in_=wo_d[:, dc, :])

                    # ---- projection: q^T, k^T ----
                    qTr = rawpool.tile([128, 2, GW], BF16, tag="qTr")
                    kTr = rawpool.tile([128, 2, GW], BF16, tag="kTr")
                    for ti, (w_sb, outT) in enumerate(
                        ((wq_sb, qTr), (wk_sb, kTr))
                    ):
                        for dc in range(2):
                            ps = ps512.tile([128, GW], F32, tag="ps512")
                            for kc in range(KC):
                                nc.tensor.matmul(
                                    ps,
                                    lhsT=w_sb[:, kc, dc * 128:(dc + 1) * 128],
                                    rhs=ht[:, kc, :],
                                    start=(kc == 0), stop=(kc == KC - 1),
                                )
                            if (ti * 2 + dc) % 2 == 0:
                                nc.scalar.copy(outT[:, dc, :], ps)
                            else:
                                nc.vector.tensor_copy(outT[:, dc, :], ps)

                    def v_proj(rbg):
                        rb = g * (GW // 128) + rbg
                        psv = ps257.tile([128, D + 1], F32, tag="ps257",
                                         name="psv")
                        for kc in range(KC):
                            nc.tensor.matmul(
                                psv[:, 0:D],
                                lhsT=ht[:, kc, rbg * 128:(rbg + 1) * 128],
                                rhs=wv_sb[:, kc, :],
                                start=(kc == 0), stop=(kc == KC - 1),
                            )
                        nc.vector.tensor_copy(v_aug[:, rb, 0:D], psv[:, 0:D])

                    # ---- v blocks 0,1: PE filler while squares run ----
                    v_proj(0)
                    v_proj(1)

                    # ---- squares (ACT) ----
                    sq_q = sqpool.tile([128, 2, GW], BF16, tag="sqq")
                    sq_k = sqpool.tile([128, 2, GW], BF16, tag="sqk")
                    for dc in range(2):
                        nc.scalar.square(sq_q[:, dc, :], qTr[:, dc, :])
                        nc.scalar.square(sq_k[:, dc, :], kTr[:, dc, :])

                    # ---- q stats: weighted broadcast sum via W2-matmul ----
                    # ps[j,t] = sum_p sq[p,t]/(1+w_p)^2  (pre-gain sum sq)
                    psq = ps512.tile([128, GW], F32, tag="ps512")
                    for dc in range(2):
                        nc.tensor.matmul(
                            psq, lhsT=w2_sb[:, dc, :], rhs=sq_q[:, dc, :],
                            start=(dc == 0), stop=(dc == 1),
                        )
                    # rstd_q*SCALING = exp(-0.5*ln((mean+eps)/SCALING^2)).
                    # ln+exp live in ONE act table together with square/copy
                    # (natural_log_exp_and_others), so no ACT_TABLE_LOad
                    # swaps and no slow DVE reciprocal.
                    rq = rstpool.tile([128, GW], F32, tag="rq")
                    nc.scalar.activation(
                        out=rq, in_=psq,
                        func=mybir.ActivationFunctionType.Ln,
                        scale=1.0 / (D * SCALING * SCALING),
                        bias=eps_q,
                    )
                    rqb = rstpool.tile([128, GW], BF16, tag="rqb")
                    nc.scalar.activation(
                        out=rqb, in_=rq,
                        func=mybir.ActivationFunctionType.Exp,
                        scale=-0.5,
                    )

                    # ---- k stats ----
                    psk = ps512.tile([128, GW], F32, tag="ps512")
                    for dc in range(2):
                        nc.tensor.matmul(
                            psk, lhsT=w2_sb[:, 2 + dc, :], rhs=sq_k[:, dc, :],
                            start=(dc == 0), stop=(dc == 1),
                        )
                    rk = rstpool.tile([128, GW], F32, tag="rk")
                    nc.scalar.activation(
                        out=rk, in_=psk,
                        func=mybir.ActivationFunctionType.Ln,
                        scale=1.0 / D, bias=eps_k,
                    )
                    rkb = rstpool.tile([128, GW], BF16, tag="rkb")
                    nc.scalar.activation(
                        out=rkb, in_=rk,
                        func=mybir.ActivationFunctionType.Exp,
                        scale=-0.5,
                    )

                    # ---- v blocks 2,3: PE filler while rstd/rope run ----
                    v_proj(2)
                    v_proj(3)

                    # ---- rope:  fin0 = (raw0*c - raw1*s) * rstd
                    # ----        fin1 = (raw1*c + raw0*s) * rstd
                    c_t = cs_sb[:, 0, gsl]
                    s_t = cs_sb[:, 1, gsl]
                    t1 = tmppool.tile([128, GW], BF16, tag="t1")
                    t2 = tmppool.tile([128, GW], BF16, tag="t2")
                    nc.vector.tensor_mul(t1, qTr[:, 0, :], c_t)
                    nc.vector.tensor_mul(t2, qTr[:, 1, :], s_t)
                    nc.vector.tensor_sub(t1, t1, t2)
                    nc.vector.tensor_mul(qTf[:, 0, gsl], t1, rqb)
                    t3 = tmppool.tile([128, GW], BF16, tag="t3")
                    t4 = tmppool.tile([128, GW], BF16, tag="t4")
                    nc.vector.tensor_mul(t3, qTr[:, 1, :], c_t)
                    nc.vector.tensor_mul(t4, qTr[:, 0, :], s_t)
                    nc.vector.tensor_add(t3, t3, t4)
                    nc.vector.tensor_mul(qTf[:, 1, gsl], t3, rqb)

                    t5 = tmppool.tile([128, GW], BF16, tag="t5")
                    t6 = tmppool.tile([128, GW], BF16, tag="t6")
                    nc.gpsimd.tensor_mul(t5, kTr[:, 0, :], c_t)
                    nc.gpsimd.tensor_mul(t6, kTr[:, 1, :], s_t)
                    nc.vector.tensor_sub(t5, t5, t6)
                    nc.vector.tensor_mul(kTf[:, 0, gsl], t5, rkb)
                    t7 = tmppool.tile([128, GW], BF16, tag="t7")
                    t8 = tmppool.tile([128, GW], BF16, tag="t8")
                    nc.gpsimd.tensor_mul(t7, kTr[:, 1, :], c_t)
                    nc.gpsimd.tensor_mul(t8, kTr[:, 0, :], s_t)
                    nc.vector.tensor_add(t7, t7, t8)
                    nc.vector.tensor_mul(kTf[:, 1, gsl], t7, rkb)

                    # ---- attention rows of the PREVIOUS group (their rope
                    # ---- finished while this group's proj matmuls ran)
                    if g >= 1:
                        for r in range(4 * (g - 1), 4 * g):
                            attn_row(r)

                # remaining rows (group 3)
                for r in range(12, 16):
                    attn_row(r)

    return nc


def make_in_maps(hidden_states, cos, sin, w_qkv, w_o, q_norm_w, k_norm_w):
    """Host-side sharding / relayout: one input map per core."""
    f32 = np.float32
    hT = _chunk_part(np.ascontiguousarray(hidden_states.T).astype(f32))
    hT = _bf16(np.ascontiguousarray(
        hT.reshape(128, KC, NG, GW).transpose(0, 2, 1, 3)))

    cosT = np.ascontiguousarray(cos.T).astype(f32)   # [D, S]
    sinT = np.ascontiguousarray(sin.T).astype(f32)
    # HF convention duplicates the freq table: rows 0:128 == rows 128:256
    cs = _bf16(np.stack([cosT[:128], sinT[:128]], axis=1))   # [128, 2, S]

    w1q = 1.0 + q_norm_w.astype(f32)
    w1k = 1.0 + k_norm_w.astype(f32)
    w2 = np.empty((128, 4, 128), f32)
    w2[:, 0, :] = (w1q[:128] ** -2)[:, None]
    w2[:, 1, :] = (w1q[128:] ** -2)[:, None]
    w2[:, 2, :] = (w1k[:128] ** -2)[:, None]
    w2[:, 3, :] = (w1k[128:] ** -2)[:, None]
    w2 = _bf16(w2)

    jj = np.arange(128)[:, None]  # key index within block (partition)
    ii = np.arange(128)[None, :]  # query index within block (free)
    mask_diag = np.where(jj <= ii, 0.0, MASK_VAL).astype(f32)
    mask_part = np.where(jj >= ii + 1, 0.0, MASK_VAL).astype(f32)
    maskT = _bf16(np.stack(
        [np.ascontiguousarray(mask_diag.T),
         np.ascontiguousarray(mask_part.T)], axis=1))        # [128, 2, 128]

    in_maps = []
    for h in range(N_CORES):
        g = h // (NH // NKV)
        wq = _bf16(_chunk_part(np.ascontiguousarray(
            w_qkv[:, h * D:(h + 1) * D]).astype(f32) * w1q[None, :]))
        wk = _bf16(_chunk_part(np.ascontiguousarray(
            w_qkv[:, NH * D + g * D: NH * D + (g + 1) * D]
        ).astype(f32) * w1k[None, :]))
        wv = _bf16(_chunk_part(np.ascontiguousarray(
            w_qkv[:, (NH + NKV) * D + g * D: (NH + NKV) * D + (g + 1) * D]
        ).astype(f32)))
        wo = _bf16(_chunk_part(np.ascontiguousarray(
            w_o[h * D:(h + 1) * D, :]).astype(f32)))
        in_maps.append({
            "hT": hT, "wq": wq, "wk": wk, "wv": wv,
            "cs": cs, "w2": w2, "wo": wo, "maskT": maskT,
        })
    return in_maps


_NC_CACHE = None


def _get_nc():
    global _NC_CACHE
    if _NC_CACHE is None:
        _NC_CACHE = split_multiwaits(build_nc())
    return _NC_CACHE


def run(inputs, trace=False, **kw):
    """Returns (full_output, BassKernelResults)."""
    nc = _get_nc()
    in_maps = make_in_maps(**inputs)
    res = run_bass_kernel_spmd(
        nc, in_maps, core_ids=list(range(N_CORES)), trace=trace, **kw
    )
    parts = [np.asarray(res.results[i]["out"], dtype=np.float32)
             for i in range(N_CORES)]
    out = np.sum(np.stack(parts, axis=0), axis=0, dtype=np.float32)
    return out, res


def kernel(**inputs) -> np.ndarray:
    out, _ = run(inputs, trace=False)
    return out


# revision 36
# speedup vs baseline: 1.0250x; 1.0250x over previous
"""Gemma3 sliding-window attention layer on 8 Trainium2 NeuronCores.

Tensor-parallel over query heads: core h computes query head h (kv head
h//2), i.e. column-parallel qkv projection, full per-head attention, and
the row-parallel slice of o_proj; the 8 partial [S, H] bf16 outputs are
summed on the host in f32 (the all-reduce / unshard step).

Layout: hidden_states transposed on the host ([H, S]) so q and k come
out of the projection directly in [d, seq] layout (what the scores
matmul needs) and v comes out natural [seq, d] (what attn@v needs).
The softmax denominator rides as a ones-column appended to v.

Schedule (software-pipelined so the PE never idles):
  proj g0 | proj g1 | attn r0-3 | proj g2 | attn r4-7 | proj g3 |
  attn r8-15
Attention rows for group g are enqueued after projection of group g+1,
so the PE chews on proj matmuls while group g's norm + rope finish on
Scalar/Vector, and on attention matmuls while the next group's DMAs
land.  DMA issue is split across the two HWDGE queues (sync + scalar)
to halve head-of-kernel issue serialization.

RMSNorm: the (1+w) gains are folded into wq/wk columns on the host; the
square-sum matmuls use per-partition-weighted lhsT (W2 = (1+w_p)^-2
replicated along free) instead of ones, so the stats still measure the
pre-gain q/k.  rstd comes from a Sqrt activation + fast-approx DVE
reciprocal.  Rope tables are then plain cos/sin (the HF duplicated-half
convention means only the first 128 rows are needed), loaded once.
"""

import numpy as np
import ml_dtypes

import concourse.bass as bass
import concourse.mybir as mybir
import concourse.tile as tile
from concourse.bass_utils import run_bass_kernel_spmd
from concourse.masks import make_identity

# ---- problem constants (hardcoded; kernel.py must be self-contained) ----
S = 2048          # sequence length
H = 2560          # hidden size
NH = 8            # query heads
NKV = 4           # kv heads
D = 256           # head dim
EPS = 1e-6
SCALING = 256.0 ** -0.5
WINDOW = 1024 - 1  # sliding window - 1

N_CORES = 8
KC = H // 128      # 20 contraction chunks for the projection
RB = S // 128      # 16 row blocks
GW = 512           # column group width
NG = S // GW       # 4 groups
MASK_VAL = -1e10
BLK_WIN = WINDOW // 128 + 1   # 8: c in [r-8, r] can contribute

F32 = mybir.dt.float32
BF16 = mybir.dt.bfloat16


def _bf16(x):
    return np.ascontiguousarray(x.astype(ml_dtypes.bfloat16))


def _chunk_part(x, p=128):
    """[c*p, n] -> [p, c, n] host relayout so it DMAs 1:1 into an SBUF tile."""
    c = x.shape[0] // p
    return np.ascontiguousarray(
        x.reshape(c, p, *x.shape[1:]).transpose(1, 0, *range(2, x.ndim + 1))
    )


def split_multiwaits(nc):
    """This toolchain's codegen allows one sync-wait slot per instruction.

    Tile emits several waits on the first consumer of multi-queue DMAs and
    on kernel-tail drains; hoist all but the last wait onto same-engine
    NoOps inserted immediately before the offending instruction (queue
    order on the engine preserves the semantics exactly).
    """
    k = 0
    for f in nc.m.functions:
        for bb in f.blocks:
            insts = bb.instructions
            if not any(i.sync_info and len(i.sync_info.on_wait) > 1
                       for i in insts):
                continue
            newlist = []
            for inst in insts:
                si = inst.sync_info
                if si is not None and len(si.on_wait) > 1:
                    for w in list(si.on_wait)[:-1]:
                        nop = mybir.InstNoOp(name=f"{inst.name}-ws{k}")
                        k += 1
                        nop.engine = inst.engine
                        nop.sync_info = mybir.SyncInfo(on_wait=[w], on_update=[])
                        newlist.append(nop)
                    inst.sync_info = mybir.SyncInfo(
                        on_wait=[list(si.on_wait)[-1]],
                        on_update=list(si.on_update))
                newlist.append(inst)
            live = bb.instructions
            live.clear()
            live.extend(newlist)
    return nc


def build_nc():
    """One-core SPMD program (all cores run this; data differs per core)."""
    nc = bass.Bass()

    hT_d = nc.declare_dram_parameter("hT", [128, NG, KC, GW], BF16, isOutput=False)
    wq_d = nc.declare_dram_parameter("wq", [128, KC, D], BF16, isOutput=False)
    wk_d = nc.declare_dram_parameter("wk", [128, KC, D], BF16, isOutput=False)
    wv_d = nc.declare_dram_parameter("wv", [128, KC, D], BF16, isOutput=False)
    cs_d = nc.declare_dram_parameter("cs", [128, 2, S], BF16, isOutput=False)
    w2_d = nc.declare_dram_parameter("w2", [128, 4, 128], BF16, isOutput=False)
    wo_d = nc.declare_dram_parameter("wo", [128, 2, H], BF16, isOutput=False)
    mk_d = nc.declare_dram_parameter("maskT", [128, 2, 128], BF16, isOutput=False)
    out_d = nc.declare_dram_parameter("out", [S, H], BF16, isOutput=True)

    with tile.TileContext(nc) as tc:
        with (
            tc.tile_pool(name="persist", bufs=1) as persist,
            tc.tile_pool(name="ps512", bufs=3, space="PSUM") as ps512,
            tc.tile_pool(name="ps257", bufs=2, space="PSUM") as ps257,
        ):
            # persistent tensors
            v_aug = persist.tile([128, RB, D + 1], BF16, tag="vaug")
            qTf = persist.tile([128, 2, S], BF16, tag="qTf")   # roped+scaled
            kTf = persist.tile([128, 2, S], BF16, tag="kTf")   # roped, unscaled
            cs_sb = persist.tile([128, 2, S], BF16, tag="cs")  # cos / sin
            w2_sb = persist.tile([128, 4, 128], BF16, tag="w2")
            maskT = persist.tile([128, 2, 128], BF16, tag="maskT")
            ident = persist.tile([128, 128], BF16, tag="ident")

            wq_sb = persist.tile([128, KC, D], BF16, tag="wq")
            wk_sb = persist.tile([128, KC, D], BF16, tag="wk")
            wv_sb = persist.tile([128, KC, D], BF16, tag="wv")
            wo_sb = persist.tile([128, 2, H], BF16, tag="wo")

            with (
                tc.tile_pool(name="ht", bufs=2) as htpool,
                tc.tile_pool(name="raw", bufs=2) as rawpool,
                tc.tile_pool(name="sq", bufs=2) as sqpool,
                tc.tile_pool(name="rst", bufs=2) as rstpool,
                tc.tile_pool(name="tmp", bufs=2) as tmppool,
                tc.tile_pool(name="probs", bufs=6) as prpool,
                tc.tile_pool(name="asmall", bufs=4) as aspool,
                tc.tile_pool(name="attnT", bufs=2) as atpool,
                tc.tile_pool(name="opool", bufs=3) as opool,
                tc.tile_pool(name="po", bufs=2, space="PSUM") as po,
                tc.tile_pool(name="psb", bufs=1, space="PSUM") as psb,
            ):
                # ---- head DMAs: first-matmul inputs first, split across
                # ---- the two HWDGE queues (sync gets wq, scalar gets hT g0)
                ht_tiles = [None] * NG
                ht_tiles[0] = htpool.tile([128, KC, GW], BF16, tag="ht",
                                          name="ht0")
                # scalar queue: dedicated to the critical hT g0 stream;
                # sync queue: wq chunks interleaved with hT's tail chunks
                ht0 = ht_tiles[0]
                nc.sync.dma_start(out=wq_sb[:, 0:2, :], in_=wq_d[:, 0:2, :])
                for c0, c1 in ((0, 2), (2, 5), (5, 8), (8, 11), (11, 14)):
                    nc.scalar.dma_start(out=ht0[:, c0:c1, :],
                                        in_=hT_d[:, 0, c0:c1, :])
                nc.sync.dma_start(out=wq_sb[:, 2:8, :], in_=wq_d[:, 2:8, :])
                nc.sync.dma_start(out=wk_sb[:, 0:4, :], in_=wk_d[:, 0:4, :])
                nc.sync.dma_start(out=ht0[:, 14:17, :],
                                  in_=hT_d[:, 0, 14:17, :])
                nc.sync.dma_start(out=wq_sb[:, 8:14, :], in_=wq_d[:, 8:14, :])
                nc.sync.dma_start(out=wk_sb[:, 4:12, :], in_=wk_d[:, 4:12, :])
                nc.sync.dma_start(out=ht0[:, 17:20, :],
                                  in_=hT_d[:, 0, 17:20, :])
                nc.sync.dma_start(out=wq_sb[:, 14:20, :],
                                  in_=wq_d[:, 14:20, :])
                nc.sync.dma_start(out=wk_sb[:, 12:20, :],
                                  in_=wk_d[:, 12:20, :])
                nc.sync.dma_start(out=wv_sb, in_=wv_d[:])
                nc.sync.dma_start(out=cs_sb, in_=cs_d[:])
                nc.sync.dma_start(out=w2_sb, in_=w2_d[:])
                nc.sync.dma_start(out=maskT, in_=mk_d[:])

                make_identity(nc, ident)
                nc.vector.memset(v_aug[:, :, D:D + 1], 1.0)
                eps_q = persist.tile([128, 1], F32, tag="eps_q")
                eps_k = persist.tile([128, 1], F32, tag="eps_k")
                nc.vector.memset(eps_q, EPS / (SCALING * SCALING))
                nc.vector.memset(eps_k, EPS)
                # warm the ln/exp activation table during the DMA-bound
                # head so the 1.3us ACT_TABLE_LOAD never blocks the
                # stats->rope chain mid-pipeline
                warm = persist.tile([128, 1], F32, tag="warm")
                nc.scalar.activation(
                    out=warm, in_=eps_k,
                    func=mybir.ActivationFunctionType.Ln)

                def attn_row(r):
                    """Scores -> exp -> attn@v -> transpose -> o_proj for
                    one 128-token query row block."""
                    cmin = max(0, r - BLK_WIN)
                    cols = list(range(cmin, r + 1))
                    ps_at = ps257.tile([128, D + 1], F32, tag="ps257")
                    chunks = [cols[i0:i0 + 4] for i0 in range(0, len(cols), 4)]
                    # all score matmuls first: exps overlap later chunks'
                    # scores, so the attn@v accumulation never stalls PE
                    psWs = []
                    for chunk in chunks:
                        psW = ps512.tile([128, 512], F32, tag="ps512")
                        for j, c in enumerate(chunk):
                            sl = slice(j * 128, (j + 1) * 128)
                            masked = (c == r) or (c == r - BLK_WIN)
                            for dc in range(2):
                                nc.tensor.matmul(
                                    psW[:, sl],
                                    lhsT=kTf[:, dc, c * 128:(c + 1) * 128],
                                    rhs=qTf[:, dc, r * 128:(r + 1) * 128],
                                    start=(dc == 0),
                                    stop=(dc == 1) and not masked,
                                )
                            if masked:
                                nc.tensor.matmul(
                                    psW[:, sl],
                                    lhsT=maskT[:, 0, :] if c == r
                                    else maskT[:, 1, :],
                                    rhs=ident,
                                    start=False, stop=True,
                                )
                        psWs.append(psW)
                    pTs = []
                    for chunk, psW in zip(chunks, psWs):
                        pT = prpool.tile([128, 512], BF16, tag="pT")
                        w = len(chunk) * 128
                        nc.scalar.activation(
                            out=pT[:, 0:w], in_=psW[:, 0:w],
                            func=mybir.ActivationFunctionType.Exp,
                        )
                        pTs.append(pT)
                    for chunk, pT in zip(chunks, pTs):
                        for j, c in enumerate(chunk):
                            sl = slice(j * 128, (j + 1) * 128)
                            nc.tensor.matmul(
                                ps_at,
                                lhsT=pT[:, sl],
                                rhs=v_aug[:, c, :],
                                start=(c == cmin), stop=(c == r),
                            )
                    rc = aspool.tile([128, 1], F32, tag="rc")
                    nc.vector.reciprocal(rc, ps_at[:, D:D + 1])
                    a_sb = aspool.tile([128, D], BF16, tag="asb")
                    nc.vector.tensor_copy(a_sb, ps_at[:, 0:D])
                    attnT = atpool.tile([128, 2, 128], BF16, tag="attnT")
                    for dc in range(2):
                        pt = psb.tile([128, 128], BF16, tag="psb")
                        nc.tensor.transpose(
                            pt, a_sb[:, dc * 128:(dc + 1) * 128], ident
                        )
                        nc.vector.tensor_copy(attnT[:, dc, :], pt)

                    # o_proj for this row; denom recip folded into the drain
                    o_sb = opool.tile([128, H], BF16, tag="osb")
                    for hc in range(H // 512):
                        ps = po.tile([128, 512], F32, tag="po")
                        for dc in range(2):
                            nc.tensor.matmul(
                                ps,
                                lhsT=attnT[:, dc, :],
                                rhs=wo_sb[:, dc, hc * 512:(hc + 1) * 512],
                                start=(dc == 0), stop=(dc == 1),
                            )
                        if hc in (0, 3):
                            nc.scalar.mul(o_sb[:, hc * 512:(hc + 1) * 512],
                                          ps, rc)
                        else:
                            with nc.allow_low_precision(
                                    reason="bf16 out slice; host sums in f32"):
                                nc.vector.tensor_scalar_mul(
                                    o_sb[:, hc * 512:(hc + 1) * 512], ps, rc)
                    for h0, h1 in ((0, 1024), (1024, 2048), (2048, H)):
                        nc.sync.dma_start(
                            out=out_d[r * 128:(r + 1) * 128, h0:h1],
                            in_=o_sb[:, h0:h1])

                for g in range(NG):
                    gsl = slice(g * GW, (g + 1) * GW)
                    ht = ht_tiles[g]
                    if g + 1 < NG:
                        # prefetch next group's hidden slice (both queues)
                        nxt = htpool.tile([128, KC, GW], BF16, tag="ht",
                                          name=f"ht{g + 1}")
                        ht_tiles[g + 1] = nxt
                        nc.sync.dma_start(out=nxt[:, 0:10, :],
                                          in_=hT_d[:, g + 1, 0:10, :])
                        nc.scalar.dma_start(out=nxt[:, 10:20, :],
                                            in_=hT_d[:, g + 1, 10:20, :])
                    if g == 1:
                        for dc in range(2):
                            nc.sync.dma_start(out=wo_sb[:, dc, :],
                                              in_=wo_d[:, dc, :])

                    # ---- projection: q^T, k^T ----
                    qTr = rawpool.tile([128, 2, GW], BF16, tag="qTr")
                    kTr = rawpool.tile([128, 2, GW], BF16, tag="kTr")
                    for ti, (w_sb, outT) in enumerate(
                        ((wq_sb, qTr), (wk_sb, kTr))
                    ):
                        for dc in range(2):
                            ps = ps512.tile([128, GW], F32, tag="ps512")
                            for kc in range(KC):
                                nc.tensor.matmul(
                                    ps,
                                    lhsT=w_sb[:, kc, dc * 128:(dc + 1) * 128],
                                    rhs=ht[:, kc, :],
                                    start=(kc == 0), stop=(kc == KC - 1),
                                )
                            if (ti * 2 + dc) % 2 == 0:
                                nc.scalar.copy(outT[:, dc, :], ps)
                            else:
                                nc.vector.tensor_copy(outT[:, dc, :], ps)

                    def v_proj(rbg):
                        rb = g * (GW // 128) + rbg
                        psv = ps257.tile([128, D + 1], F32, tag="ps257",
                                         name="psv")
                        for kc in range(KC):
                            nc.tensor.matmul(
                                psv[:, 0:D],
                                lhsT=ht[:, kc, rbg * 128:(rbg + 1) * 128],
                                rhs=wv_sb[:, kc, :],
                                start=(kc == 0), stop=(kc == KC - 1),
                            )
                        nc.vector.tensor_copy(v_aug[:, rb, 0:D], psv[:, 0:D])

                    # ---- v blocks 0,1: PE filler while squares run ----
                    v_proj(0)
                    v_proj(1)

                    # ---- squares (ACT) ----
                    sq_q = sqpool.tile([128, 2, GW], BF16, tag="sqq")
                    sq_k = sqpool.tile([128, 2, GW], BF16, tag="sqk")
                    for dc in range(2):
                        nc.scalar.square(sq_q[:, dc, :], qTr[:, dc, :])
                        nc.scalar.square(sq_k[:, dc, :], kTr[:, dc, :])

                    # ---- q stats: weighted broadcast sum via W2-matmul ----
                    # ps[j,t] = sum_p sq[p,t]/(1+w_p)^2  (pre-gain sum sq)
                    psq = ps512.tile([128, GW], F32, tag="ps512")
                    for dc in range(2):
                        nc.tensor.matmul(
                            psq, lhsT=w2_sb[:, dc, :], rhs=sq_q[:, dc, :],
                            start=(dc == 0), stop=(dc == 1),
                        )
                    # rstd_q*SCALING = exp(-0.5*ln((mean+eps)/SCALING^2)).
                    # ln+exp live in ONE act table together with square/copy
                    # (natural_log_exp_and_others), so no ACT_TABLE_LOad
                    # swaps and no slow DVE reciprocal.
                    rq = rstpool.tile([128, GW], F32, tag="rq")
                    nc.scalar.activation(
                        out=rq, in_=psq,
                        func=mybir.ActivationFunctionType.Ln,
                        scale=1.0 / (D * SCALING * SCALING),
                        bias=eps_q,
                    )
                    rqb = rstpool.tile([128, GW], BF16, tag="rqb")
                    nc.scalar.activation(
                        out=rqb, in_=rq,
                        func=mybir.ActivationFunctionType.Exp,
                        scale=-0.5,
                    )

                    # ---- k stats ----
                    psk = ps512.tile([128, GW], F32, tag="ps512")
                    for dc in range(2):
                        nc.tensor.matmul(
                            psk, lhsT=w2_sb[:, 2 + dc, :], rhs=sq_k[:, dc, :],
                            start=(dc == 0), stop=(dc == 1),
                        )
                    rk = rstpool.tile([128, GW], F32, tag="rk")
                    nc.scalar.activation(
                        out=rk, in_=psk,
                        func=mybir.ActivationFunctionType.Ln,
                        scale=1.0 / D, bias=eps_k,
                    )
                    rkb = rstpool.tile([128, GW], BF16, tag="rkb")
                    nc.scalar.activation(
                        out=rkb, in_=rk,
                        func=mybir.ActivationFunctionType.Exp,
                        scale=-0.5,
                    )

                    # ---- v blocks 2,3: PE filler while rstd/rope run ----
                    v_proj(2)
                    v_proj(3)

                    # ---- rope:  fin0 = (raw0*c - raw1*s) * rstd
                    # ----        fin1 = (raw1*c + raw0*s) * rstd
                    c_t = cs_sb[:, 0, gsl]
                    s_t = cs_sb[:, 1, gsl]
                    t1 = tmppool.tile([128, GW], BF16, tag="t1")
                    t2 = tmppool.tile([128, GW], BF16, tag="t2")
                    nc.vector.tensor_mul(t1, qTr[:, 0, :], c_t)
                    nc.vector.tensor_mul(t2, qTr[:, 1, :], s_t)
                    nc.vector.tensor_sub(t1, t1, t2)
                    nc.vector.tensor_mul(qTf[:, 0, gsl], t1, rqb)
                    t3 = tmppool.tile([128, GW], BF16, tag="t3")
                    t4 = tmppool.tile([128, GW], BF16, tag="t4")
                    nc.vector.tensor_mul(t3, qTr[:, 1, :], c_t)
                    nc.vector.tensor_mul(t4, qTr[:, 0, :], s_t)
                    nc.vector.tensor_add(t3, t3, t4)
                    nc.vector.tensor_mul(qTf[:, 1, gsl], t3, rqb)

                    t5 = tmppool.tile([128, GW], BF16, tag="t5")
                    t6 = tmppool.tile([128, GW], BF16, tag="t6")
                    nc.gpsimd.tensor_mul(t5, kTr[:, 0, :], c_t)
                    nc.gpsimd.tensor_mul(t6, kTr[:, 1, :], s_t)
                    nc.vector.tensor_sub(t5, t5, t6)
                    nc.vector.tensor_mul(kTf[:, 0, gsl], t5, rkb)
                    t7 = tmppool.tile([128, GW], BF16, tag="t7")
                    t8 = tmppool.tile([128, GW], BF16, tag="t8")
                    nc.gpsimd.tensor_mul(t7, kTr[:, 1, :], c_t)
                    nc.gpsimd.tensor_mul(t8, kTr[:, 0, :], s_t)
                    nc.vector.tensor_add(t7, t7, t8)
                    nc.vector.tensor_mul(kTf[:, 1, gsl], t7, rkb)

                    # ---- attention rows of the PREVIOUS group (their rope
                    # ---- finished while this group's proj matmuls ran)
                    if g >= 1:
                        for r in range(4 * (g - 1), 4 * g):
                            attn_row(r)

                # remaining rows (group 3)
                for r in range(12, 16):
                    attn_row(r)

    return nc


def make_in_maps(hidden_states, cos, sin, w_qkv, w_o, q_norm_w, k_norm_w):
    """Host-side sharding / relayout: one input map per core."""
    f32 = np.float32
    hT = _chunk_part(np.ascontiguousarray(hidden_states.T).astype(f32))
    hT = _bf16(np.ascontiguousarray(
        hT.reshape(128, KC, NG, GW).transpose(0, 2, 1, 3)))

    cosT = np.ascontiguousarray(cos.T).astype(f32)   # [D, S]
    sinT = np.ascontiguousarray(sin.T).astype(f32)
    # HF convention duplicates the freq table: rows 0:128 == rows 128:256
    cs = _bf16(np.stack([cosT[:128], sinT[:128]], axis=1))   # [128, 2, S]

    w1q = 1.0 + q_norm_w.astype(f32)
    w1k = 1.0 + k_norm_w.astype(f32)
    w2 = np.empty((128, 4, 128), f32)
    w2[:, 0, :] = (w1q[:128] ** -2)[:, None]
    w2[:, 1, :] = (w1q[128:] ** -2)[:, None]
    w2[:, 2, :] = (w1k[:128] ** -2)[:, None]
    w2[:, 3, :] = (w1k[128:] ** -2)[:, None]
    w2 = _bf16(w2)

    jj = np.arange(128)[:, None]  # key index within block (partition)
    ii = np.arange(128)[None, :]  # query index within block (free)
    mask_diag = np.where(jj <= ii, 0.0, MASK_VAL).astype(f32)
    mask_part = np.where(jj >= ii + 1, 0.0, MASK_VAL).astype(f32)
    maskT = _bf16(np.stack(
        [np.ascontiguousarray(mask_diag.T),
         np.ascontiguousarray(mask_part.T)], axis=1))        # [128, 2, 128]

    in_maps = []
    for h in range(N_CORES):
        g = h // (NH // NKV)
        wq = _bf16(_chunk_part(np.ascontiguousarray(
            w_qkv[:, h * D:(h + 1) * D]).astype(f32) * w1q[None, :]))
        wk = _bf16(_chunk_part(np.ascontiguousarray(
            w_qkv[:, NH * D + g * D: NH * D + (g + 1) * D]
        ).astype(f32) * w1k[None, :]))
        wv = _bf16(_chunk_part(np.ascontiguousarray(
            w_qkv[:, (NH + NKV) * D + g * D: (NH + NKV) * D + (g + 1) * D]
        ).astype(f32)))
        wo = _bf16(_chunk_part(np.ascontiguousarray(
            w_o[h * D:(h + 1) * D, :]).astype(f32)))
        in_maps.append({
            "hT": hT, "wq": wq, "wk": wk, "wv": wv,
            "cs": cs, "w2": w2, "wo": wo, "maskT": maskT,
        })
    return in_maps


_NC_CACHE = None


def _get_nc():
    global _NC_CACHE
    if _NC_CACHE is None:
        _NC_CACHE = split_multiwaits(build_nc())
    return _NC_CACHE


def run(inputs, trace=False, **kw):
    """Returns (full_output, BassKernelResults)."""
    nc = _get_nc()
    in_maps = make_in_maps(**inputs)
    res = run_bass_kernel_spmd(
        nc, in_maps, core_ids=list(range(N_CORES)), trace=trace, **kw
    )
    parts = [np.asarray(res.results[i]["out"], dtype=np.float32)
             for i in range(N_CORES)]
    out = np.sum(np.stack(parts, axis=0), axis=0, dtype=np.float32)
    return out, res


def kernel(**inputs) -> np.ndarray:
    out, _ = run(inputs, trace=False)
    return out


# revision 37
# speedup vs baseline: 1.0291x; 1.0040x over previous
"""Gemma3 sliding-window attention layer on 8 Trainium2 NeuronCores.

Tensor-parallel over query heads: core h computes query head h (kv head
h//2), i.e. column-parallel qkv projection, full per-head attention, and
the row-parallel slice of o_proj; the 8 partial [S, H] bf16 outputs are
summed on the host in f32 (the all-reduce / unshard step).

Layout: hidden_states transposed on the host ([H, S]) so q and k come
out of the projection directly in [d, seq] layout (what the scores
matmul needs) and v comes out natural [seq, d] (what attn@v needs).
The softmax denominator rides as a ones-column appended to v.

Schedule (software-pipelined so the PE never idles):
  proj g0 | proj g1 | attn r0-3 | proj g2 | attn r4-7 | proj g3 |
  attn r8-15
Attention rows for group g are enqueued after projection of group g+1,
so the PE chews on proj matmuls while group g's norm + rope finish on
Scalar/Vector, and on attention matmuls while the next group's DMAs
land.  DMA issue is split across the two HWDGE queues (sync + scalar)
to halve head-of-kernel issue serialization.

RMSNorm: the (1+w) gains are folded into wq/wk columns on the host; the
square-sum matmuls use per-partition-weighted lhsT (W2 = (1+w_p)^-2
replicated along free) instead of ones, so the stats still measure the
pre-gain q/k.  rstd comes from a Sqrt activation + fast-approx DVE
reciprocal.  Rope tables are then plain cos/sin (the HF duplicated-half
convention means only the first 128 rows are needed), loaded once.
"""

import numpy as np
import ml_dtypes

import concourse.bass as bass
import concourse.mybir as mybir
import concourse.tile as tile
from concourse.bass_utils import run_bass_kernel_spmd
from concourse.masks import make_identity

# ---- problem constants (hardcoded; kernel.py must be self-contained) ----
S = 2048          # sequence length
H = 2560          # hidden size
NH = 8            # query heads
NKV = 4           # kv heads
D = 256           # head dim
EPS = 1e-6
SCALING = 256.0 ** -0.5
WINDOW = 1024 - 1  # sliding window - 1

N_CORES = 8
KC = H // 128      # 20 contraction chunks for the projection
RB = S // 128      # 16 row blocks
GW = 512           # column group width
NG = S // GW       # 4 groups
MASK_VAL = -1e10
BLK_WIN = WINDOW // 128 + 1   # 8: c in [r-8, r] can contribute

F32 = mybir.dt.float32
BF16 = mybir.dt.bfloat16


def _bf16(x):
    return np.ascontiguousarray(x.astype(ml_dtypes.bfloat16))


def _chunk_part(x, p=128):
    """[c*p, n] -> [p, c, n] host relayout so it DMAs 1:1 into an SBUF tile."""
    c = x.shape[0] // p
    return np.ascontiguousarray(
        x.reshape(c, p, *x.shape[1:]).transpose(1, 0, *range(2, x.ndim + 1))
    )


def split_multiwaits(nc):
    """This toolchain's codegen allows one sync-wait slot per instruction.

    Tile emits several waits on the first consumer of multi-queue DMAs and
    on kernel-tail drains; hoist all but the last wait onto same-engine
    NoOps inserted immediately before the offending instruction (queue
    order on the engine preserves the semantics exactly).
    """
    k = 0
    for f in nc.m.functions:
        for bb in f.blocks:
            insts = bb.instructions
            if not any(i.sync_info and len(i.sync_info.on_wait) > 1
                       for i in insts):
                continue
            newlist = []
            for inst in insts:
                si = inst.sync_info
                if si is not None and len(si.on_wait) > 1:
                    for w in list(si.on_wait)[:-1]:
                        nop = mybir.InstNoOp(name=f"{inst.name}-ws{k}")
                        k += 1
                        nop.engine = inst.engine
                        nop.sync_info = mybir.SyncInfo(on_wait=[w], on_update=[])
                        newlist.append(nop)
                    inst.sync_info = mybir.SyncInfo(
                        on_wait=[list(si.on_wait)[-1]],
                        on_update=list(si.on_update))
                newlist.append(inst)
            live = bb.instructions
            live.clear()
            live.extend(newlist)
    return nc


def build_nc():
    """One-core SPMD program (all cores run this; data differs per core)."""
    nc = bass.Bass()

    hT_d = nc.declare_dram_parameter("hT", [128, NG, KC, GW], BF16, isOutput=False)
    wq_d = nc.declare_dram_parameter("wq", [128, KC, D], BF16, isOutput=False)
    wk_d = nc.declare_dram_parameter("wk", [128, KC, D], BF16, isOutput=False)
    wv_d = nc.declare_dram_parameter("wv", [128, KC, D], BF16, isOutput=False)
    cs_d = nc.declare_dram_parameter("cs", [128, 2, S], BF16, isOutput=False)
    w2_d = nc.declare_dram_parameter("w2", [128, 4, 128], BF16, isOutput=False)
    wo_d = nc.declare_dram_parameter("wo", [128, 2, H], BF16, isOutput=False)
    mk_d = nc.declare_dram_parameter("maskT", [128, 2, 128], BF16, isOutput=False)
    out_d = nc.declare_dram_parameter("out", [S, H], BF16, isOutput=True)

    with tile.TileContext(nc) as tc:
        with (
            tc.tile_pool(name="persist", bufs=1) as persist,
            tc.tile_pool(name="ps512", bufs=3, space="PSUM") as ps512,
            tc.tile_pool(name="ps257", bufs=2, space="PSUM") as ps257,
        ):
            # persistent tensors
            v_aug = persist.tile([128, RB, D + 1], BF16, tag="vaug")
            qTf = persist.tile([128, 2, S], BF16, tag="qTf")   # roped+scaled
            kTf = persist.tile([128, 2, S], BF16, tag="kTf")   # roped, unscaled
            cs_sb = persist.tile([128, 2, S], BF16, tag="cs")  # cos / sin
            w2_sb = persist.tile([128, 4, 128], BF16, tag="w2")
            maskT = persist.tile([128, 2, 128], BF16, tag="maskT")
            ident = persist.tile([128, 128], BF16, tag="ident")

            wq_sb = persist.tile([128, KC, D], BF16, tag="wq")
            wk_sb = persist.tile([128, KC, D], BF16, tag="wk")
            wv_sb = persist.tile([128, KC, D], BF16, tag="wv")
            wo_sb = persist.tile([128, 2, H], BF16, tag="wo")

            with (
                tc.tile_pool(name="ht", bufs=2) as htpool,
                tc.tile_pool(name="raw", bufs=2) as rawpool,
                tc.tile_pool(name="sq", bufs=2) as sqpool,
                tc.tile_pool(name="rst", bufs=2) as rstpool,
                tc.tile_pool(name="tmp", bufs=2) as tmppool,
                tc.tile_pool(name="probs", bufs=6) as prpool,
                tc.tile_pool(name="asmall", bufs=4) as aspool,
                tc.tile_pool(name="attnT", bufs=2) as atpool,
                tc.tile_pool(name="opool", bufs=4) as opool,
                tc.tile_pool(name="po", bufs=2, space="PSUM") as po,
                tc.tile_pool(name="psb", bufs=1, space="PSUM") as psb,
            ):
                # ---- head DMAs: first-matmul inputs first, split across
                # ---- the two HWDGE queues (sync gets wq, scalar gets hT g0)
                ht_tiles = [None] * NG
                ht_tiles[0] = htpool.tile([128, KC, GW], BF16, tag="ht",
                                          name="ht0")
                # scalar queue: dedicated to the critical hT g0 stream;
                # sync queue: wq chunks interleaved with hT's tail chunks
                ht0 = ht_tiles[0]
                nc.sync.dma_start(out=wq_sb[:, 0:2, :], in_=wq_d[:, 0:2, :])
                for c0, c1 in ((0, 2), (2, 5), (5, 8), (8, 11), (11, 14)):
                    nc.scalar.dma_start(out=ht0[:, c0:c1, :],
                                        in_=hT_d[:, 0, c0:c1, :])
                nc.sync.dma_start(out=wq_sb[:, 2:8, :], in_=wq_d[:, 2:8, :])
                nc.sync.dma_start(out=wk_sb[:, 0:4, :], in_=wk_d[:, 0:4, :])
                nc.sync.dma_start(out=ht0[:, 14:17, :],
                                  in_=hT_d[:, 0, 14:17, :])
                nc.sync.dma_start(out=wq_sb[:, 8:14, :], in_=wq_d[:, 8:14, :])
                nc.sync.dma_start(out=wk_sb[:, 4:12, :], in_=wk_d[:, 4:12, :])
                nc.sync.dma_start(out=ht0[:, 17:20, :],
                                  in_=hT_d[:, 0, 17:20, :])
                nc.sync.dma_start(out=wq_sb[:, 14:20, :],
                                  in_=wq_d[:, 14:20, :])
                nc.sync.dma_start(out=wk_sb[:, 12:20, :],
                                  in_=wk_d[:, 12:20, :])
                nc.sync.dma_start(out=wv_sb, in_=wv_d[:])
                nc.sync.dma_start(out=cs_sb, in_=cs_d[:])
                nc.sync.dma_start(out=w2_sb, in_=w2_d[:])
                nc.sync.dma_start(out=maskT, in_=mk_d[:])

                make_identity(nc, ident)
                nc.vector.memset(v_aug[:, :, D:D + 1], 1.0)
                eps_q = persist.tile([128, 1], F32, tag="eps_q")
                eps_k = persist.tile([128, 1], F32, tag="eps_k")
                nc.vector.memset(eps_q, EPS / (SCALING * SCALING))
                nc.vector.memset(eps_k, EPS)
                # warm the ln/exp activation table during the DMA-bound
                # head so the 1.3us ACT_TABLE_LOAD never blocks the
                # stats->rope chain mid-pipeline
                warm = persist.tile([128, 1], F32, tag="warm")
                nc.scalar.activation(
                    out=warm, in_=eps_k,
                    func=mybir.ActivationFunctionType.Ln)

                def attn_row(r):
                    """Scores -> exp -> attn@v -> transpose -> o_proj for
                    one 128-token query row block."""
                    cmin = max(0, r - BLK_WIN)
                    cols = list(range(cmin, r + 1))
                    ps_at = ps257.tile([128, D + 1], F32, tag="ps257")
                    chunks = [cols[i0:i0 + 4] for i0 in range(0, len(cols), 4)]
                    # all score matmuls first: exps overlap later chunks'
                    # scores, so the attn@v accumulation never stalls PE
                    psWs = []
                    for chunk in chunks:
                        psW = ps512.tile([128, 512], F32, tag="ps512")
                        for j, c in enumerate(chunk):
                            sl = slice(j * 128, (j + 1) * 128)
                            masked = (c == r) or (c == r - BLK_WIN)
                            for dc in range(2):
                                nc.tensor.matmul(
                                    psW[:, sl],
                                    lhsT=kTf[:, dc, c * 128:(c + 1) * 128],
                                    rhs=qTf[:, dc, r * 128:(r + 1) * 128],
                                    start=(dc == 0),
                                    stop=(dc == 1) and not masked,
                                )
                            if masked:
                                nc.tensor.matmul(
                                    psW[:, sl],
                                    lhsT=maskT[:, 0, :] if c == r
                                    else maskT[:, 1, :],
                                    rhs=ident,
                                    start=False, stop=True,
                                )
                        psWs.append(psW)
                    pTs = []
                    for chunk, psW in zip(chunks, psWs):
                        pT = prpool.tile([128, 512], BF16, tag="pT")
                        w = len(chunk) * 128
                        nc.scalar.activation(
                            out=pT[:, 0:w], in_=psW[:, 0:w],
                            func=mybir.ActivationFunctionType.Exp,
                        )
                        pTs.append(pT)
                    for chunk, pT in zip(chunks, pTs):
                        for j, c in enumerate(chunk):
                            sl = slice(j * 128, (j + 1) * 128)
                            nc.tensor.matmul(
                                ps_at,
                                lhsT=pT[:, sl],
                                rhs=v_aug[:, c, :],
                                start=(c == cmin), stop=(c == r),
                            )
                    rc = aspool.tile([128, 1], F32, tag="rc")
                    nc.vector.reciprocal(rc, ps_at[:, D:D + 1])
                    a_sb = aspool.tile([128, D], BF16, tag="asb")
                    nc.vector.tensor_copy(a_sb, ps_at[:, 0:D])
                    attnT = atpool.tile([128, 2, 128], BF16, tag="attnT")
                    for dc in range(2):
                        pt = psb.tile([128, 128], BF16, tag="psb")
                        nc.tensor.transpose(
                            pt, a_sb[:, dc * 128:(dc + 1) * 128], ident
                        )
                        nc.vector.tensor_copy(attnT[:, dc, :], pt)

                    # o_proj for this row; denom recip folded into the drain
                    o_sb = opool.tile([128, H], BF16, tag="osb")
                    for hc in range(H // 512):
                        ps = po.tile([128, 512], F32, tag="po")
                        for dc in range(2):
                            nc.tensor.matmul(
                                ps,
                                lhsT=attnT[:, dc, :],
                                rhs=wo_sb[:, dc, hc * 512:(hc + 1) * 512],
                                start=(dc == 0), stop=(dc == 1),
                            )
                        if hc in (0, 3):
                            nc.scalar.mul(o_sb[:, hc * 512:(hc + 1) * 512],
                                          ps, rc)
                        else:
                            with nc.allow_low_precision(
                                    reason="bf16 out slice; host sums in f32"):
                                nc.vector.tensor_scalar_mul(
                                    o_sb[:, hc * 512:(hc + 1) * 512], ps, rc)
                    for h0, h1 in ((0, 1024), (1024, 2048), (2048, H)):
                        nc.sync.dma_start(
                            out=out_d[r * 128:(r + 1) * 128, h0:h1],
                            in_=o_sb[:, h0:h1])

                for g in range(NG):
                    gsl = slice(g * GW, (g + 1) * GW)
                    ht = ht_tiles[g]
                    if g + 1 < NG:
                        # prefetch next group's hidden slice (both queues)
                        nxt = htpool.tile([128, KC, GW], BF16, tag="ht",
                                          name=f"ht{g + 1}")
                        ht_tiles[g + 1] = nxt
                        nc.sync.dma_start(out=nxt[:, 0:10, :],
                                          in_=hT_d[:, g + 1, 0:10, :])
                        nc.scalar.dma_start(out=nxt[:, 10:20, :],
                                            in_=hT_d[:, g + 1, 10:20, :])
                    if g == 1:
                        for dc in range(2):
                            nc.sync.dma_start(out=wo_sb[:, dc, :],
                                              in_=wo_d[:, dc, :])

                    # ---- projection: q^T, k^T ----
                    qTr = rawpool.tile([128, 2, GW], BF16, tag="qTr")
                    kTr = rawpool.tile([128, 2, GW], BF16, tag="kTr")
                    for ti, (w_sb, outT) in enumerate(
                        ((wq_sb, qTr), (wk_sb, kTr))
                    ):
                        for dc in range(2):
                            ps = ps512.tile([128, GW], F32, tag="ps512")
                            for kc in range(KC):
                                nc.tensor.matmul(
                                    ps,
                                    lhsT=w_sb[:, kc, dc * 128:(dc + 1) * 128],
                                    rhs=ht[:, kc, :],
                                    start=(kc == 0), stop=(kc == KC - 1),
                                )
                            if (ti * 2 + dc) % 2 == 0:
                                nc.scalar.copy(outT[:, dc, :], ps)
                            else:
                                nc.vector.tensor_copy(outT[:, dc, :], ps)

                    def v_proj(rbg):
                        rb = g * (GW // 128) + rbg
                        psv = ps257.tile([128, D + 1], F32, tag="ps257",
                                         name="psv")
                        for kc in range(KC):
                            nc.tensor.matmul(
                                psv[:, 0:D],
                                lhsT=ht[:, kc, rbg * 128:(rbg + 1) * 128],
                                rhs=wv_sb[:, kc, :],
                                start=(kc == 0), stop=(kc == KC - 1),
                            )
                        nc.vector.tensor_copy(v_aug[:, rb, 0:D], psv[:, 0:D])

                    # ---- v blocks 0,1: PE filler while squares run ----
                    v_proj(0)
                    v_proj(1)

                    # ---- squares (ACT) ----
                    sq_q = sqpool.tile([128, 2, GW], BF16, tag="sqq")
                    sq_k = sqpool.tile([128, 2, GW], BF16, tag="sqk")
                    for dc in range(2):
                        nc.scalar.square(sq_q[:, dc, :], qTr[:, dc, :])
                        nc.scalar.square(sq_k[:, dc, :], kTr[:, dc, :])

                    # ---- q stats: weighted broadcast sum via W2-matmul ----
                    # ps[j,t] = sum_p sq[p,t]/(1+w_p)^2  (pre-gain sum sq)
                    psq = ps512.tile([128, GW], F32, tag="ps512")
                    for dc in range(2):
                        nc.tensor.matmul(
                            psq, lhsT=w2_sb[:, dc, :], rhs=sq_q[:, dc, :],
                            start=(dc == 0), stop=(dc == 1),
                        )
                    # rstd_q*SCALING = exp(-0.5*ln((mean+eps)/SCALING^2)).
                    # ln+exp live in ONE act table together with square/copy
                    # (natural_log_exp_and_others), so no ACT_TABLE_LOad
                    # swaps and no slow DVE reciprocal.
                    rq = rstpool.tile([128, GW], F32, tag="rq")
                    nc.scalar.activation(
                        out=rq, in_=psq,
                        func=mybir.ActivationFunctionType.Ln,
                        scale=1.0 / (D * SCALING * SCALING),
                        bias=eps_q,
                    )
                    rqb = rstpool.tile([128, GW], BF16, tag="rqb")
                    nc.scalar.activation(
                        out=rqb, in_=rq,
                        func=mybir.ActivationFunctionType.Exp,
                        scale=-0.5,
                    )

                    # ---- k stats ----
                    psk = ps512.tile([128, GW], F32, tag="ps512")
                    for dc in range(2):
                        nc.tensor.matmul(
                            psk, lhsT=w2_sb[:, 2 + dc, :], rhs=sq_k[:, dc, :],
                            start=(dc == 0), stop=(dc == 1),
                        )
                    rk = rstpool.tile([128, GW], F32, tag="rk")
                    nc.scalar.activation(
                        out=rk, in_=psk,
                        func=mybir.ActivationFunctionType.Ln,
                        scale=1.0 / D, bias=eps_k,
                    )
                    rkb = rstpool.tile([128, GW], BF16, tag="rkb")
                    nc.scalar.activation(
                        out=rkb, in_=rk,
                        func=mybir.ActivationFunctionType.Exp,
                        scale=-0.5,
                    )

                    # ---- v blocks 2,3: PE filler while rstd/rope run ----
                    v_proj(2)
                    v_proj(3)

                    # ---- rope:  fin0 = (raw0*c - raw1*s) * rstd
                    # ----        fin1 = (raw1*c + raw0*s) * rstd
                    c_t = cs_sb[:, 0, gsl]
                    s_t = cs_sb[:, 1, gsl]
                    t1 = tmppool.tile([128, GW], BF16, tag="t1")
                    t2 = tmppool.tile([128, GW], BF16, tag="t2")
                    nc.vector.tensor_mul(t1, qTr[:, 0, :], c_t)
                    nc.vector.tensor_mul(t2, qTr[:, 1, :], s_t)
                    nc.vector.tensor_sub(t1, t1, t2)
                    nc.vector.tensor_mul(qTf[:, 0, gsl], t1, rqb)
                    t3 = tmppool.tile([128, GW], BF16, tag="t3")
                    t4 = tmppool.tile([128, GW], BF16, tag="t4")
                    nc.vector.tensor_mul(t3, qTr[:, 1, :], c_t)
                    nc.vector.tensor_mul(t4, qTr[:, 0, :], s_t)
                    nc.vector.tensor_add(t3, t3, t4)
                    nc.vector.tensor_mul(qTf[:, 1, gsl], t3, rqb)

                    t5 = tmppool.tile([128, GW], BF16, tag="t5")
                    t6 = tmppool.tile([128, GW], BF16, tag="t6")
                    nc.gpsimd.tensor_mul(t5, kTr[:, 0, :], c_t)
                    nc.gpsimd.tensor_mul(t6, kTr[:, 1, :], s_t)
                    nc.vector.tensor_sub(t5, t5, t6)
                    nc.vector.tensor_mul(kTf[:, 0, gsl], t5, rkb)
                    t7 = tmppool.tile([128, GW], BF16, tag="t7")
                    t8 = tmppool.tile([128, GW], BF16, tag="t8")
                    nc.gpsimd.tensor_mul(t7, kTr[:, 1, :], c_t)
                    nc.gpsimd.tensor_mul(t8, kTr[:, 0, :], s_t)
                    nc.vector.tensor_add(t7, t7, t8)
                    nc.vector.tensor_mul(kTf[:, 1, gsl], t7, rkb)

                    # ---- attention rows of the PREVIOUS group (their rope
                    # ---- finished while this group's proj matmuls ran)
                    if g >= 1:
                        for r in range(4 * (g - 1), 4 * g):
                            attn_row(r)

                # remaining rows (group 3)
                for r in range(12, 16):
                    attn_row(r)

    return nc


def make_in_maps(hidden_states, cos, sin, w_qkv, w_o, q_norm_w, k_norm_w):
    """Host-side sharding / relayout: one input map per core."""
    f32 = np.float32
    hT = _chunk_part(np.ascontiguousarray(hidden_states.T).astype(f32))
    hT = _bf16(np.ascontiguousarray(
        hT.reshape(128, KC, NG, GW).transpose(0, 2, 1, 3)))

    cosT = np.ascontiguousarray(cos.T).astype(f32)   # [D, S]
    sinT = np.ascontiguousarray(sin.T).astype(f32)
    # HF convention duplicates the freq table: rows 0:128 == rows 128:256
    cs = _bf16(np.stack([cosT[:128], sinT[:128]], axis=1))   # [128, 2, S]

    w1q = 1.0 + q_norm_w.astype(f32)
    w1k = 1.0 + k_norm_w.astype(f32)
    w2 = np.empty((128, 4, 128), f32)
    w2[:, 0, :] = (w1q[:128] ** -2)[:, None]
    w2[:, 1, :] = (w1q[128:] ** -2)[:, None]
    w2[:, 2, :] = (w1k[:128] ** -2)[:, None]
    w2[:, 3, :] = (w1k[128:] ** -2)[:, None]
    w2 = _bf16(w2)

    jj = np.arange(128)[:, None]  # key index within block (partition)
    ii = np.arange(128)[None, :]  # query index within block (free)
    mask_diag = np.where(jj <= ii, 0.0, MASK_VAL).astype(f32)
    mask_part = np.where(jj >= ii + 1, 0.0, MASK_VAL).astype(f32)
    maskT = _bf16(np.stack(
        [np.ascontiguousarray(mask_diag.T),
         np.ascontiguousarray(mask_part.T)], axis=1))        # [128, 2, 128]

    in_maps = []
    for h in range(N_CORES):
        g = h // (NH // NKV)
        wq = _bf16(_chunk_part(np.ascontiguousarray(
            w_qkv[:, h * D:(h + 1) * D]).astype(f32) * w1q[None, :]))
        wk = _bf16(_chunk_part(np.ascontiguousarray(
            w_qkv[:, NH * D + g * D: NH * D + (g + 1) * D]
        ).astype(f32) * w1k[None, :]))
        wv = _bf16(_chunk_part(np.ascontiguousarray(
            w_qkv[:, (NH + NKV) * D + g * D: (NH + NKV) * D + (g + 1) * D]
        ).astype(f32)))
        wo = _bf16(_chunk_part(np.ascontiguousarray(
            w_o[h * D:(h + 1) * D, :]).astype(f32)))
        in_maps.append({
            "hT": hT, "wq": wq, "wk": wk, "wv": wv,
            "cs": cs, "w2": w2, "wo": wo, "maskT": maskT,
        })
    return in_maps


_NC_CACHE = None


def _get_nc():
    global _NC_CACHE
    if _NC_CACHE is None:
        _NC_CACHE = split_multiwaits(build_nc())
    return _NC_CACHE


def run(inputs, trace=False, **kw):
    """Returns (full_output, BassKernelResults)."""
    nc = _get_nc()
    in_maps = make_in_maps(**inputs)
    res = run_bass_kernel_spmd(
        nc, in_maps, core_ids=list(range(N_CORES)), trace=trace, **kw
    )
    parts = [np.asarray(res.results[i]["out"], dtype=np.float32)
             for i in range(N_CORES)]
    out = np.sum(np.stack(parts, axis=0), axis=0, dtype=np.float32)
    return out, res


def kernel(**inputs) -> np.ndarray:
    out, _ = run(inputs, trace=False)
    return out
